# revision 1
# baseline (speedup 1.0000x reference)
"""AlphaRotatedGIoULoss on 8 TRN2 NeuronCores.

Data-parallel: 500000 box pairs sharded 62500/core, laid out as
(125 partitions x 500 boxes). Per-box rotated-GIoU via a branchless
line-integral intersection (slab clipping in each box's axis-aligned
frame + a frame-change correction term), so no sorting/gather is needed.

Final version (199.5us -> ~114us on HW vs the session-measured baseline):
- heavy elementwise chain in fp16 (DVE 2x_1p mode = 2x throughput), with
  geometry pre-scaled by 1/16 (folded into existing scale factors) so
  all products stay in fp16 range; reciprocal slab planes clamped to
  +-3e4 so 0*inf NaNs cannot occur.
- all 128 partitions used (FB=489); rows 62500..62591 are host-padded
  concentric 16px/8px axis-aligned boxes whose giou is exactly 1/64
  (subtracted on the host). Identical pad boxes would hit the
  coincident-boundary degeneracy of the Green's-theorem formulation.
- host repack: angle/wh columns cast to fp16, xy quantized to int16
  units of 1/32 px (diffs stay fp16-exact; dequant folded into the trig
  scale); all three shipped pre-shuffled into the exact SBUF tile layout
  so each partition line is one contiguous DMA descriptor. DMA order
  ang -> wh -> xy unblocks the Sin chain early.
- point symmetry (corner e2,e3 = 2*dX - e0,e1; slab roots 2,3 = m +
  roots 0,1) exploited with stride-0 broadcast APs to merge op pairs
  into single wide DVE passes. Enclosing-box extents via
  half_extent_x = |wc|+|hs|. Output: per-core sum(giou); host 1 - s/N.
"""
import sys
import numpy as np

for _p in ("/opt/trn_rl_repo", "/root/.axon_site/_ro/trn_rl_repo"):
    if _p not in sys.path:
        sys.path.insert(0, _p)

N_CORES = 8
N_TOTAL = 500000
N_CORE = N_TOTAL // N_CORES   # 62500
P = 128                       # all partitions
FB = 489                      # boxes per partition row (128*489 = 62592)
NPAD = P * FB                 # per-core padded count (92 identity pad boxes)
SW = 2 * FB                   # stacked width (both halves)
PI_2 = 1.5707963267948966
SC = 1.0 / 16.0               # global geometry scale (power of 2, exact)
XQ = 32.0                     # xy fixed-point scale (int16 units = px/32)
XSC = SC / XQ                 # folds the xy dequant into the trig scale
CL = 30000.0                  # fp16-safe clamp for reciprocal planes

_CACHE = {}


def _build():
    import concourse.bass as bass
    import concourse.bacc as bacc
    import concourse.tile as tile
    from concourse import mybir

    f32 = mybir.dt.float32
    f16 = mybir.dt.float16
    i16 = mybir.dt.int16
    AF = mybir.ActivationFunctionType
    OP = mybir.AluOpType
    AXL = mybir.AxisListType
    import os
    debug = bool(os.environ.get("K_DEBUG"))
    nc = bacc.Bacc(None, target_bir_lowering=False)
    ang_d = nc.declare_dram_parameter("ang", [P, 2 * FB], f16, isOutput=False)
    wh_d = nc.declare_dram_parameter("wh", [P, 4 * FB], f16, isOutput=False)
    xy_d = nc.declare_dram_parameter("xy", [P, 4 * FB], i16, isOutput=False)
    out_d = nc.declare_dram_parameter("out", [P, 1], f32, isOutput=True)
    dbg_d = None
    if debug:
        dbg_d = nc.declare_dram_parameter("dbg", [4, P, FB], f32, isOutput=True)

    V = nc.vector
    S = nc.scalar

    def vtt(out, a, b, op):
        V.tensor_tensor(out, a, b, op)

    def vts(out, in_, s1, s2, op0, op1=None):
        if op1 is None:
            V.tensor_scalar(out, in_, s1, None, op0)
        else:
            V.tensor_scalar(out, in_, s1, s2, op0, op1)

    def bce(apv, n=2, axis=1):
        # stride-0 broadcast: insert a [0, n] dim at `axis` (after partition)
        ap_l = [list(d) for d in apv.ap]
        ap_l.insert(axis, [0, n])
        return bass.AP(apv.tensor, apv.offset, ap_l)

    from contextlib import ExitStack

    with tile.TileContext(nc) as tc:
        with (
            tc.tile_pool(name="pre", bufs=1) as pre,
            tc.tile_pool(name="small", bufs=1) as sm,
            ExitStack() as stack,
        ):
            io = stack.enter_context(tc.tile_pool(name="io", bufs=1))
            angT = io.tile([P, 2 * FB], f16, tag="angT")
            whT = io.tile([P, 4 * FB], f16, tag="whT")
            xyT = io.tile([P, 4 * FB], i16, tag="xyT")
            pio2 = sm.tile([P, 1], f32, tag="pio2")
            V.memset(pio2[:], PI_2)
            # 1-elem warm-up: loads the Sin ACT table while the DMA runs
            warm = sm.tile([P, 1], f32, tag="warm")
            S.activation(warm[:], pio2[:], AF.Sin)
            angV = angT[:].rearrange("p (h f) -> p h f", h=2)
            whV = whT[:].rearrange("p (c f) -> p c f", c=4)   # w1,w2,h1,h2
            xyV = xyT[:].rearrange("p (c f) -> p c f", c=4)   # x1,x2,y1,y2
            # host pre-shuffles inputs into these exact SBUF layouts, so each
            # partition line is one fully-contiguous DMA descriptor.
            # angles first (small, unblocks the Sin chain), then wh, then xy
            nc.sync.dma_start(out=angT[:], in_=ang_d[:])
            nc.sync.dma_start(out=whT[:], in_=wh_d[:])
            nc.sync.dma_start(out=xyT[:], in_=xy_d[:])

            class SP:
                def __init__(self, name, dt=f16, w=FB, k=2):
                    self.w = w
                    self.t = pre.tile([P, k * w], dt, tag=name)

                def full(self):
                    return self.t[:]

                def h(self, i):
                    return self.t[:, i * self.w:(i + 1) * self.w]

                def v3(self):     # (P, 2, w) stacked view
                    return self.t[:].rearrange("p (h f) -> p h f", h=2)

            # paired tiles (P, 2, SW): two SW-wide planes side by side
            ddS = SP("ddS", w=SW)       # [ddx | ddy]
            cdsd = SP("cdsd", w=SW)     # [cd | sd]
            wcws = SP("wcws", w=SW)     # [wc | ws]
            hchs = SP("hchs", w=SW)     # [hc | hs]
            aP1, aP2 = SP("aP1", w=SW), SP("aP2", w=SW)
            dx16, dy16 = SP("dx16"), SP("dy16")
            dlt, dltw = SP("dlt", f32), SP("dltw", f32)
            cS, sS = SP("cS"), SP("sS")
            csS, ssS = SP("csS"), SP("ssS")
            dX, dY = SP("dX"), SP("dY")
            dXm, dYm = SP("dXm"), SP("dYm")
            whS, hhS = SP("whS"), SP("hhS")
            g0x, g0y, n1, n2 = SP("g0x"), SP("g0y"), SP("n1"), SP("n2")
            Wc, Hc, nWc, nHc = SP("Wc"), SP("Hc"), SP("nWc"), SP("nHc")
            exP, eyP = SP("exP"), SP("eyP")
            rp32a, rp32b = SP("rp32a", f32), SP("rp32b", f32)
            ddxS, ddyS = ddS.v3()[:, 0], ddS.v3()[:, 1]     # (P, SW) each
            cdS, sdS = cdsd.v3()[:, 0], cdsd.v3()[:, 1]
            wcF, wsF = wcws.v3()[:, 0], wcws.v3()[:, 1]
            hcF, hsF = hchs.v3()[:, 0], hchs.v3()[:, 1]

            def hviews(flat):     # (P, 2, FB) of an (P, SW) flat view
                return flat.rearrange("p (h f) -> p h f", h=2)

            # persistent pre-signed clamped reciprocal planes, (P, 2e, 2h, FB)
            rIX = pre.tile([P, 2 * SW], f16, tag="rIX")
            rIY = pre.tile([P, 2 * SW], f16, tag="rIY")
            rIXe = rIX[:].rearrange("p (e h f) -> p e h f", e=2, h=2)
            rIYe = rIY[:].rearrange("p (e h f) -> p e h f", e=2, h=2)

            # ---- pre-pass, angle part (only needs angT) ----
            vtt(dlt.h(0), angV[:, 0], angV[:, 1], OP.subtract)    # a1-a2 (f32)
            vts(dlt.h(1), dlt.h(0), -1.0, None, OP.mult)
            S.activation(cS.h(0), angV[:, 1], AF.Sin, bias=pio2[:])  # c2
            S.activation(cS.h(1), angV[:, 0], AF.Sin, bias=pio2[:])  # c1
            S.activation(sS.h(0), angV[:, 1], AF.Sin)                # s2
            S.activation(sS.h(1), angV[:, 0], AF.Sin)                # s1
            S.activation(sdS, dlt.full(), AF.Sin)                    # [sd|-sd]
            # cos(dlt) = sin(dlt + pi/2); wrap into [-pi, pi] first
            V.add_range_wrap(dltw.full(), dlt.full(), PI_2, 3.141592653589793,
                             6.283185307179586)
            S.activation(cdS, dltw.full(), AF.Sin)                   # [cd|cd]
            # scaled trig copies carry geometry scale + xy dequant into dX/dY
            vts(csS.full(), cS.full(), XSC, None, OP.mult)
            vts(ssS.full(), sS.full(), XSC, None, OP.mult)

            # ---- pre-pass, wh part ----
            vts(whS.full(), whV[:, 0:2], 0.5 * SC, None, OP.mult)  # [w1|w2]/32
            vts(hhS.full(), whV[:, 2:4], 0.5 * SC, None, OP.mult)
            # [wc|ws] = whS * [cd|sd];  [hc|hs] = hhS * [cd|sd]
            cdsd4 = cdsd.t[:].rearrange("p (c h f) -> p c h f", c=2, h=2)
            vtt(wcws.t[:].rearrange("p (c h f) -> p c h f", c=2, h=2),
                bce(whS.v3()), cdsd4, OP.mult)
            vtt(hchs.t[:].rearrange("p (c h f) -> p c h f", c=2, h=2),
                bce(hhS.v3()), cdsd4, OP.mult)
            vtt(g0x.full(), wcF, hsF, OP.subtract)
            vtt(g0y.full(), wsF, hcF, OP.add)
            vtt(n1.full(), wcF, hsF, OP.add)              # -g1x
            vtt(n2.full(), hcF, wsF, OP.subtract)         # g1y
            # clip half-extents of the fixed box, /16 (+neg)
            vts(Wc.h(0), whV[:, 1], 0.5 * SC, None, OP.mult)
            vts(Wc.h(1), whV[:, 0], 0.5 * SC, None, OP.mult)
            vts(Hc.h(0), whV[:, 3], 0.5 * SC, None, OP.mult)
            vts(Hc.h(1), whV[:, 2], 0.5 * SC, None, OP.mult)
            vts(nWc.full(), Wc.full(), -1.0, None, OP.mult)
            vts(nHc.full(), Hc.full(), -1.0, None, OP.mult)
            # moving-box bbox half-extents: ex = |wc|+|hs|, ey = |ws|+|hc|
            S.activation(aP1.full(), wcws.full(), AF.Abs)   # [|wc| | |ws|]
            S.activation(aP2.full(), hchs.full(), AF.Abs)   # [|hc| | |hs|]
            vtt(exP.full(), aP1.v3()[:, 0], aP2.v3()[:, 1], OP.add)
            vtt(eyP.full(), aP1.v3()[:, 1], aP2.v3()[:, 0], OP.add)
            # pre-signed reciprocal slab planes: rIX e0 = -1/(2wc),
            # e1 = +1/(2hs); rIY e0 = -1/(2ws), e1 = -1/(2hc).
            # clamp to +-CL then fp16 so 0*inf NaNs cannot occur.
            for (dst, src, sgn, rp) in (
                (rIXe[:, 0], wcF, -1.0, rp32a),
                (rIXe[:, 1], hsF, 1.0, rp32b),
                (rIYe[:, 0], wsF, -1.0, rp32a),
                (rIYe[:, 1], hcF, -1.0, rp32b),
            ):
                vts(rp.full(), src, 2.0 * sgn, 1e-20 * sgn, OP.mult, OP.add)
                V.reciprocal_approx_fast(out=rp.full(), in_=rp.full())
                vts(dst, rp.v3(), CL, -CL, OP.min, OP.max)
            # union0 = (w1h1 + w2h2)/1024; the *4 to reach the /256 scale of
            # inter is folded into the final union STT
            u01 = sm.tile([P, SW], f16, tag="u01")
            union0 = sm.tile([P, FB], f32, tag="union0")
            vtt(u01[:], whS.full(), hhS.full(), OP.mult)
            u013 = u01[:].rearrange("p (h f) -> p h f", h=2)
            vtt(union0[:], u013[:, 0], u013[:, 1], OP.add)

            # ---- pre-pass, xy part (lands last) ----
            vtt(hviews(ddxS)[:, 0], xyV[:, 0], xyV[:, 1], OP.subtract)  # x1-x2
            vts(hviews(ddxS)[:, 1], hviews(ddxS)[:, 0], -1.0, None, OP.mult)
            vtt(hviews(ddyS)[:, 0], xyV[:, 2], xyV[:, 3], OP.subtract)
            vts(hviews(ddyS)[:, 1], hviews(ddyS)[:, 0], -1.0, None, OP.mult)
            vts(dx16.full(), ddxS, XSC, None, OP.mult)
            vts(dy16.full(), ddyS, XSC, None, OP.mult)
            # delta = R^T * (center difference)/16, stacked:
            # P1 = [csS*ddx | csS*ddy], P2 = [ssS*ddx | ssS*ddy]
            ddc = ddS.t[:].rearrange("p (c h f) -> p c h f", c=2, h=2)
            vtt(aP1.t[:].rearrange("p (c h f) -> p c h f", c=2, h=2),
                bce(csS.v3()), ddc, OP.mult)
            vtt(aP2.t[:].rearrange("p (c h f) -> p c h f", c=2, h=2),
                bce(ssS.v3()), ddc, OP.mult)
            vtt(dX.full(), aP1.v3()[:, 0], aP2.v3()[:, 1], OP.add)
            vtt(dY.full(), aP1.v3()[:, 1], aP2.v3()[:, 0], OP.subtract)
            vts(dXm.full(), dX.full(), 2.0, None, OP.mult)        # 2*dx
            vts(dYm.full(), dY.full(), 2.0, None, OP.mult)

            # input tiles no longer needed: free the io pool
            stack.close()
            hv = stack.enter_context(tc.tile_pool(name="heavy", bufs=1))

            def E(tile4):     # (P, 4, 2, FB) edge/half view of 4*SW tile
                return tile4[:].rearrange("p (e h f) -> p e h f", e=4, h=2)

            AXt = hv.tile([P, 4 * SW], f16, tag="AXt")
            AYt = hv.tile([P, 4 * SW], f16, tag="AYt")
            DRX = hv.tile([P, 4 * SW], f16, tag="DRX")
            DRY = hv.tile([P, 4 * SW], f16, tag="DRY")
            Ut = hv.tile([P, 4 * SW], f16, tag="Ut")
            Vt = hv.tile([P, 4 * SW], f16, tag="Vt")
            NPt = hv.tile([P, 4 * SW], f16, tag="NPt")
            TLX = hv.tile([P, 4 * SW], f16, tag="TLX")

            # corners: e0,e1 explicit; e2,e3 = 2*dX - e0,e1 (point symmetry)
            vtt(E(AXt)[:, 0], dX.v3(), g0x.v3(), OP.add)
            vtt(E(AXt)[:, 1], dX.v3(), n1.v3(), OP.subtract)
            vtt(E(AXt)[:, 2:4], bce(dXm.v3()), E(AXt)[:, 0:2], OP.subtract)
            vtt(E(AYt)[:, 0], dY.v3(), g0y.v3(), OP.add)
            vtt(E(AYt)[:, 1], dY.v3(), n2.v3(), OP.add)
            vtt(E(AYt)[:, 2:4], bce(dYm.v3()), E(AYt)[:, 0:2], OP.subtract)

            # ---- enclosing rect (bbox in each frame, min of the two) ----
            exm = sm.tile([P, SW], f16, tag="exm")
            exn = sm.tile([P, SW], f16, tag="exn")
            exs = sm.tile([P, SW], f16, tag="exs")
            eys = sm.tile([P, SW], f16, tag="eys")
            ex3 = exm[:].rearrange("p (h f) -> p h f", h=2)
            en3 = exn[:].rearrange("p (h f) -> p h f", h=2)
            es3 = exs[:].rearrange("p (h f) -> p h f", h=2)
            ey3 = eys[:].rearrange("p (h f) -> p h f", h=2)
            for ext, d3, clamp, dst3 in ((exP, dX, Wc, es3), (eyP, dY, Hc, ey3)):
                vtt(ex3, d3.v3(), ext.v3(), OP.add)               # dX + ex
                vtt(en3, ext.v3(), d3.v3(), OP.subtract)          # ex - dX
                vtt(ex3, ex3, clamp.v3(), OP.max)
                vtt(en3, en3, clamp.v3(), OP.max)
                vtt(dst3, ex3, en3, OP.add)                       # extent
            vtt(exs[:], exs[:], eys[:], OP.mult)                  # areaC stacked
            area_c = sm.tile([P, FB], f32, tag="area_c")
            vtt(area_c[:], es3[:, 0], es3[:, 1], OP.min)

            HW2 = 2 * SW

            def H01(t4):
                return t4[:, 0:HW2]

            def H23(t4):
                return t4[:, HW2:2 * HW2]

            # ---- slab clip, x axis, edges 0,1 (2,3 via point symmetry:
            # roots(edge2) = m + roots(edge0), m = dXm*rIX) ----
            vtt(E(Ut)[:, 0:2], bce(nWc.v3()), E(AXt)[:, 0:2], OP.subtract)
            vtt(E(Vt)[:, 0:2], bce(Wc.v3()), E(AXt)[:, 0:2], OP.subtract)
            vtt(H01(Ut), H01(Ut), rIX[:], OP.mult)                 # ta01
            vtt(H01(Vt), H01(Vt), rIX[:], OP.mult)                 # tb01
            vtt(H01(TLX), H01(Ut), H01(Vt), OP.min)                # tlo01
            vtt(H01(Ut), H01(Ut), H01(Vt), OP.max)                 # thi01
            vtt(E(Vt)[:, 0:2], bce(dXm.v3()), rIXe, OP.mult)       # m01
            vtt(H23(TLX), H01(Vt), H01(TLX), OP.add)               # tlo23
            vtt(H23(Ut), H01(Vt), H01(Ut), OP.add)                 # thi23
            # ---- slab clip, y axis, edges 0,1 ----
            vtt(E(Vt)[:, 0:2], bce(nHc.v3()), E(AYt)[:, 0:2], OP.subtract)
            vtt(E(NPt)[:, 0:2], bce(Hc.v3()), E(AYt)[:, 0:2], OP.subtract)
            vtt(H01(Vt), H01(Vt), rIY[:], OP.mult)                 # ta01_y
            vtt(H01(NPt), H01(NPt), rIY[:], OP.mult)               # tb01_y
            vtt(H01(DRX), H01(Vt), H01(NPt), OP.min)               # tlo01_y
            vtt(H01(Vt), H01(Vt), H01(NPt), OP.max)                # thi01_y
            vtt(E(NPt)[:, 0:2], bce(dYm.v3()), rIYe, OP.mult)      # m01_y
            vtt(H23(DRX), H01(NPt), H01(DRX), OP.add)              # tlo23_y
            vtt(H23(Vt), H01(NPt), H01(Vt), OP.add)                # thi23_y
            # ---- interval intersect, dt ----
            # t0 = max(tlo_x, tlo_y, 0); t1 = min(thi_x, thi_y, 1)
            vtt(TLX[:], TLX[:], DRX[:], OP.max)
            vts(TLX[:], TLX[:], 0.0, None, OP.max)
            vtt(Ut[:], Ut[:], Vt[:], OP.min)
            vts(Ut[:], Ut[:], 1.0, None, OP.min)
            vtt(TLX[:], Ut[:], TLX[:], OP.subtract)                # t1-t0
            S.activation(TLX[:], TLX[:], AF.Relu)                  # dt
            # ---- direction planes (on Scalar), cross(a,d), pieces ----
            for dst, srcs in (
                (DRX, ((wcF, -2.0), (hsF, 2.0), (wcF, 2.0), (hsF, -2.0))),
                (DRY, ((wsF, -2.0), (hcF, -2.0), (wsF, 2.0), (hcF, 2.0))),
            ):
                d4 = E(dst)
                for e, (src, sc) in enumerate(srcs):
                    S.activation(d4[:, e], hviews(src), AF.Copy, scale=sc)
            vtt(Vt[:], AXt[:], DRY[:], OP.mult)                    # ax*dy
            vtt(NPt[:], AYt[:], DRX[:], OP.mult)                   # ay*dx
            vtt(Vt[:], Vt[:], NPt[:], OP.subtract)                 # cad
            vtt(Ut[:], TLX[:], Vt[:], OP.mult)                     # pieces

            # ---- piece sum (stacked), SA correction (frame-B half) ----
            psS = sm.tile([P, SW], f16, tag="psS")
            ps3 = psS[:].rearrange("p (h f) -> p h f", h=2)
            u4 = E(Ut)
            vtt(ps3, u4[:, 0], u4[:, 1], OP.add)
            vtt(es3, u4[:, 2], u4[:, 3], OP.add)                   # reuse exs
            vtt(ps3, ps3, es3, OP.add)
            dt4 = E(TLX)
            sax = sm.tile([P, FB], f16, tag="sax")
            say = sm.tile([P, FB], f16, tag="say")
            sau = sm.tile([P, FB], f16, tag="sau")
            sav = sm.tile([P, FB], f16, tag="sav")
            st1 = sm.tile([P, FB], f16, tag="st1")
            vtt(sau[:], dt4[:, 2, 0], dt4[:, 0, 0], OP.subtract)
            vtt(sav[:], dt4[:, 3, 0], dt4[:, 1, 0], OP.subtract)
            dx4, dy4 = E(DRX), E(DRY)
            vtt(sax[:], dx4[:, 2, 0], sau[:], OP.mult)
            vtt(st1[:], dx4[:, 3, 0], sav[:], OP.mult)
            vtt(sax[:], sax[:], st1[:], OP.add)
            vtt(say[:], dy4[:, 2, 0], sau[:], OP.mult)
            vtt(st1[:], dy4[:, 3, 0], sav[:], OP.mult)
            vtt(say[:], say[:], st1[:], OP.add)
            # corr = ddy/16*(c2*sax - s2*say) - ddx/16*(s2*sax + c2*say)
            c2v = cS.h(0)
            s2v = sS.h(0)
            rsx = sm.tile([P, FB], f16, tag="rsx")
            rsy = sm.tile([P, FB], f16, tag="rsy")
            vtt(rsx[:], c2v, sax[:], OP.mult)
            vtt(st1[:], s2v, say[:], OP.mult)
            vtt(rsx[:], rsx[:], st1[:], OP.subtract)
            vtt(rsy[:], s2v, sax[:], OP.mult)
            vtt(st1[:], c2v, say[:], OP.mult)
            vtt(rsy[:], rsy[:], st1[:], OP.add)
            inter16 = sm.tile([P, FB], f16, tag="inter16")
            vtt(inter16[:], dy16.h(0), rsx[:], OP.mult)
            vtt(st1[:], dx16.h(0), rsy[:], OP.mult)
            vtt(inter16[:], inter16[:], st1[:], OP.subtract)       # corr
            vtt(inter16[:], inter16[:], ps3[:, 0], OP.add)
            vtt(inter16[:], inter16[:], ps3[:, 1], OP.add)
            inter = sm.tile([P, FB], f32, tag="inter")
            S.activation(inter[:], inter16[:], AF.Relu, scale=0.5)  # inter area

            # ---- final loss (fp32) ----
            union = sm.tile([P, FB], f32, tag="union")
            fr1 = sm.tile([P, FB], f32, tag="fr1")
            iou = sm.tile([P, FB], f32, tag="iou")
            rr = sm.tile([P, FB], f32, tag="rr")
            lsa = sm.tile([P, 1], f32, tag="lsa")
            # union = 4*union0 - inter  (the *4 restores the /256 scale)
            V.scalar_tensor_tensor(union[:], union0[:], 4.0, inter[:],
                                   OP.mult, OP.subtract)
            V.reciprocal_approx_fast(out=fr1[:], in_=union[:])
            vtt(iou[:], inter[:], fr1[:], OP.mult)
            vts(iou[:], iou[:], 1e-6, None, OP.max)
            V.reciprocal_approx_fast(out=fr1[:], in_=area_c[:])
            vtt(fr1[:], union[:], fr1[:], OP.mult)
            vts(rr[:], fr1[:], -1.0, 1.0, OP.mult, OP.add)         # 1 - u/ac
            vtt(fr1[:], iou[:], iou[:], OP.mult)                   # iou^2
            vtt(fr1[:], fr1[:], iou[:], OP.mult)                   # iou^3
            vtt(iou[:], rr[:], rr[:], OP.mult)                     # rr^2
            vtt(iou[:], iou[:], rr[:], OP.mult)                    # rr^3
            vtt(fr1[:], fr1[:], iou[:], OP.subtract)               # giou
            V.tensor_reduce(lsa[:], fr1[:], AXL.X, OP.add)         # sum giou
            if debug:
                nc.sync.dma_start(out=dbg_d[0], in_=fr1[:])
                nc.sync.dma_start(out=dbg_d[1], in_=inter[:])
                nc.sync.dma_start(out=dbg_d[2], in_=union[:])
                nc.sync.dma_start(out=dbg_d[3], in_=area_c[:])
            nc.sync.dma_start(out=out_d[:], in_=lsa[:])

    nc.finalize()
    return nc


def _get_nc():
    if "nc" not in _CACHE:
        _CACHE["nc"] = _build()
    return _CACHE["nc"]


def _repack(pred, target):
    """Per-core input repack: planar rows so every SBUF slice is packed.
    ang/wh in fp16; xy quantized to int16 units of 1/32 px (diffs <= ~1500
    units stay exact in fp16). Rows beyond N_CORE are padded with identity
    boxes (w=h=16, a=0, same centers) whose giou is exactly 1."""
    in_maps = []
    for i in range(N_CORES):
        sl = slice(i * N_CORE, (i + 1) * N_CORE)
        p, t = pred[sl], target[sl]
        ang = np.zeros((2, NPAD), np.float16)
        ang[0, :N_CORE] = p[:, 4]
        ang[1, :N_CORE] = t[:, 4]
        # pads: concentric axis-aligned 16-box (pred) vs 8-box (target):
        # iou = 1/4, rr = 0 -> giou = 1/64 exactly (all fp16-exact values;
        # identical boxes would hit the coincident-boundary degeneracy)
        wh = np.empty((4, NPAD), np.float16)
        wh[0, N_CORE:] = 16.0
        wh[1, N_CORE:] = 8.0
        wh[2, N_CORE:] = 16.0
        wh[3, N_CORE:] = 8.0
        wh[0, :N_CORE] = p[:, 2]
        wh[1, :N_CORE] = t[:, 2]
        wh[2, :N_CORE] = p[:, 3]
        wh[3, :N_CORE] = t[:, 3]
        xy = np.full((4, NPAD), 16384, np.int16)
        for r, col in enumerate((p[:, 0], t[:, 0], p[:, 1], t[:, 1])):
            xy[r, :N_CORE] = np.clip(np.rint(col * XQ), 0, 32767).astype(np.int16)
        # shuffle each (k, NPAD) row-plane set into the SBUF tile layout
        # (P, k*FB): partition-contiguous single-descriptor DMA lines
        def lay(a):
            k = a.shape[0]
            return np.ascontiguousarray(
                a.reshape(k, P, FB).transpose(1, 0, 2).reshape(P, k * FB))
        in_maps.append({"ang": lay(ang), "wh": lay(wh), "xy": lay(xy)})
    return in_maps


def kernel(pred, target):
    from concourse.bass_utils import run_bass_kernel_spmd

    pred = np.ascontiguousarray(np.asarray(pred, dtype=np.float32))
    target = np.ascontiguousarray(np.asarray(target, dtype=np.float32))
    nc = _get_nc()
    in_maps = _repack(pred, target)
    res = run_bass_kernel_spmd(nc, in_maps, core_ids=list(range(N_CORES)))
    gsum = np.float64(0.0)
    for i in range(N_CORES):
        gsum += np.asarray(res.results[i]["out"], dtype=np.float64).sum()
    # subtract the exact giou (=1/64) of the concentric pad boxes
    gsum -= float((NPAD - N_CORE) * N_CORES) * 0.015625
    # loss = mean(1 - giou) = 1 - sum(giou)/N
    return np.float32(1.0 - gsum / N_TOTAL)



# revision 8
# speedup vs baseline: 2.2179x; 2.2179x over previous
"""AlphaRotatedGIoULoss on 8 TRN2 NeuronCores.

Data-parallel: 500000 box pairs sharded 62500/core, laid out as
(128 partitions x 489 boxes). Per-box rotated-GIoU via a branchless
line-integral intersection (slab clipping in each box's axis-aligned
frame + a frame-change correction term), so no sorting/gather is needed.

Restructured v2 (vs the 113us session baseline):
- slab roots in center +- half-window form: t = c -+ |Wc*r| with
  pre-negated reciprocal planes, killing the per-edge min/max and subs;
  c23 = dXm*rN - c01 by point symmetry. x and y axes fused into 8u ops.
- cross(corner_e, dir_e)/2 = +-(cross(center,dir)/2) + wh/1024 (the wh
  term is the union's u01 tile), so the full corner planes e2/e3, the
  4SW direction planes, and the 8 ACT copies that built them are gone.
  Everything downstream runs at half-cad scale (final Relu scale 1.0).
- all four reciprocal planes merged into one wide RECIPROCAL_APPROX_FAST
  plus two clamp/cast passes; enclosing-rect x/y stacked into 4u ops.
- tail packs [iou|rr] so one ACT Square + one mult does both cubes.
Heavy chain is fp16 (DVE 2x mode), geometry pre-scaled by 1/16; scratch
tiles are re-used across phases to stay inside SBUF.
"""
import sys
import numpy as np

for _p in ("/opt/trn_rl_repo", "/root/.axon_site/_ro/trn_rl_repo"):
    if _p not in sys.path:
        sys.path.insert(0, _p)

N_CORES = 8
N_TOTAL = 500000
N_CORE = N_TOTAL // N_CORES   # 62500
P = 128                       # all partitions
FB = 489                      # boxes per partition row (128*489 = 62592)
NPAD = P * FB                 # per-core padded count (92 identity pad boxes)
SW = 2 * FB                   # stacked width (both halves)
PI_2 = 1.5707963267948966
SC = 1.0 / 16.0               # global geometry scale (power of 2, exact)
XQ = 32.0                     # xy fixed-point scale (int16 units = px/32)
XSC = SC / XQ                 # folds the xy dequant into the trig scale
CL = 30000.0                  # fp16-safe clamp for reciprocal planes

_CACHE = {}


def _build():
    import concourse.bass as bass
    import concourse.bacc as bacc
    import concourse.tile as tile
    from concourse import mybir

    f32 = mybir.dt.float32
    f16 = mybir.dt.float16
    i16 = mybir.dt.int16
    AF = mybir.ActivationFunctionType
    OP = mybir.AluOpType
    AXL = mybir.AxisListType
    import os
    debug = bool(os.environ.get("K_DEBUG"))
    nc = bacc.Bacc(None, target_bir_lowering=False)
    ang_d = nc.declare_dram_parameter("ang", [P, 2 * FB], f16, isOutput=False)
    wh_d = nc.declare_dram_parameter("wh", [P, 4 * FB], f16, isOutput=False)
    xy_d = nc.declare_dram_parameter("xy", [P, 4 * FB], i16, isOutput=False)
    out_d = nc.declare_dram_parameter("out", [P, 1], f32, isOutput=True)
    dbg_d = None
    if debug:
        dbg_d = nc.declare_dram_parameter("dbg", [4, P, FB], f32, isOutput=True)

    V = nc.vector
    S = nc.scalar

    def vtt(out, a, b, op):
        V.tensor_tensor(out, a, b, op)

    def vts(out, in_, s1, s2, op0, op1=None):
        if op1 is None:
            V.tensor_scalar(out, in_, s1, None, op0)
        else:
            V.tensor_scalar(out, in_, s1, s2, op0, op1)

    def bce(apv, n=2, axis=1):
        # stride-0 broadcast: insert a [0, n] dim at `axis` (after partition)
        ap_l = [list(d) for d in apv.ap]
        ap_l.insert(axis, [0, n])
        return bass.AP(apv.tensor, apv.offset, ap_l)

    def v2(ap, h=2):
        return ap.rearrange("p (h f) -> p h f", h=h)

    from contextlib import ExitStack

    with tile.TileContext(nc) as tc:
        with (
            tc.tile_pool(name="pre", bufs=1) as pre,
            tc.tile_pool(name="small", bufs=1) as sm,
            ExitStack() as stack,
        ):
            io = stack.enter_context(tc.tile_pool(name="io", bufs=1))
            angT = io.tile([P, 2 * FB], f16, tag="angT")
            whT = io.tile([P, 4 * FB], f16, tag="whT")
            xyT = io.tile([P, 4 * FB], i16, tag="xyT")
            pio2 = sm.tile([P, 1], f32, tag="pio2")
            V.memset(pio2[:], PI_2)
            # 1-elem warm-up: loads the Sin ACT table while the DMA runs
            warm = sm.tile([P, 1], f32, tag="warm")
            S.activation(warm[:], pio2[:], AF.Sin)
            angV = angT[:].rearrange("p (h f) -> p h f", h=2)
            whV = whT[:].rearrange("p (c f) -> p c f", c=4)   # w1,w2,h1,h2
            xyV = xyT[:].rearrange("p (c f) -> p c f", c=4)   # x1,x2,y1,y2
            # host pre-shuffles inputs into these exact SBUF layouts, so each
            # partition line is one fully-contiguous DMA descriptor.
            # angles first (small, unblocks the Sin chain), then wh, then xy
            nc.sync.dma_start(out=angT[:], in_=ang_d[:])
            nc.sync.dma_start(out=whT[:], in_=wh_d[:])
            nc.sync.dma_start(out=xyT[:], in_=xy_d[:])

            def T(name, units, dt=f16):
                # `units` in FB-widths
                return pre.tile([P, units * FB], dt, name=name, tag=name)

            # --- tiles (persistent + phase-reused scratch) ---
            dlt, dltw = T("dlt", 2, f32), T("dltw", 2, f32)
            cdsd = T("cdsd", 4)                   # [cd | sd] planes
            cS, sS = T("cS", 2), T("sS", 2)       # [c2|c1], [s2|s1]
            csS, ssS = T("csS", 2), T("ssS", 2)
            WH = T("WH", 4)                       # [whS | hhS]
            WC2 = T("WC2", 4)                     # [wc | hc]
            WS2 = T("WS2", 4)                     # [ws | hs]
            WHc = T("WHc", 4)                     # [Wc | Hc] clip half-extents
            GN = T("GN", 8)                       # [g0x | n1 | g0y | n2]
            AXY = T("AXY", 8)                     # corners [ax0|ax1|ay0|ay1]
            RP = T("RP", 8, f32)                  # recip staging [wc|hs|ws|hc]
            RN = T("RN", 8)                       # [rNX(e0,e1) | rNY(e0,e1)]
            MW = T("MW", 8)                       # [Wc*rNX | Hc*rNY] -> scratch
            AW = T("AW", 8)                       # |MW|
            CXY = T("CXY", 16)                    # [c01xy | c23xy]
            TLO = T("TLO", 16)                    # tlo; later T0/CAD/PC
            THI = T("THI", 16)                    # thi; later T1/TD/DT
            DXY = T("DXY", 6)                     # [dX | dY | -dX]
            DM = T("DM", 4)                       # [2dX | 2dY]
            ddS = T("ddS", 4)                     # [ddx | ddy]
            dx16, dy16 = T("dx16", 1), T("dy16", 1)
            aP1, aP2 = T("aP1", 4), T("aP2", 4)
            EP = T("EP", 4)                       # [exP | eyP]
            u01 = sm.tile([P, SW], f16, tag="u01")
            union0 = sm.tile([P, FB], f32, tag="union0")
            area_c = sm.tile([P, FB], f32, tag="area_c")

            dXv = DXY[:, 0 * SW:1 * SW]
            dYv = DXY[:, 1 * SW:2 * SW]
            nXv = DXY[:, 2 * SW:3 * SW]
            wcF = WC2[:, 0:SW]
            hcF = WC2[:, SW:2 * SW]
            wsF = WS2[:, 0:SW]
            hsF = WS2[:, SW:2 * SW]
            cdS = cdsd[:, 0:SW]
            sdS = cdsd[:, SW:2 * SW]

            # ---- pre-pass, angle part (only needs angT) ----
            dlt3 = v2(dlt[:])
            vtt(dlt3[:, 0], angV[:, 0], angV[:, 1], OP.subtract)  # a1-a2 (f32)
            vts(dlt3[:, 1], dlt3[:, 0], -1.0, None, OP.mult)
            S.activation(cS[:, 0:FB], angV[:, 1], AF.Sin, bias=pio2[:])   # c2
            S.activation(cS[:, FB:SW], angV[:, 0], AF.Sin, bias=pio2[:])  # c1
            S.activation(sS[:, 0:FB], angV[:, 1], AF.Sin)                 # s2
            S.activation(sS[:, FB:SW], angV[:, 0], AF.Sin)                # s1
            S.activation(sdS, dlt[:], AF.Sin)                    # [sd|-sd]
            # cos(dlt) = sin(dlt + pi/2); wrap into [-pi, pi] first
            V.add_range_wrap(dltw[:], dlt[:], PI_2, 3.141592653589793,
                             6.283185307179586)
            S.activation(cdS, dltw[:], AF.Sin)                   # [cd|cd]
            # scaled trig copies carry geometry scale + xy dequant into dX/dY
            vts(csS[:], cS[:], XSC, None, OP.mult)
            vts(ssS[:], sS[:], XSC, None, OP.mult)

            # ---- pre-pass, wh part ----
            vts(WH[:, 0:SW], whV[:, 0:2], 0.5 * SC, None, OP.mult)       # whS
            vts(WH[:, SW:2 * SW], whV[:, 2:4], 0.5 * SC, None, OP.mult)  # hhS
            WH3 = v2(WH[:])
            # [wc|hc] = [whS|hhS]*cd ; [ws|hs] = [whS|hhS]*sd
            vtt(v2(WC2[:]), WH3, bce(cdS), OP.mult)
            vtt(v2(WS2[:]), WH3, bce(sdS), OP.mult)
            # corner offsets: g0x = wc-hs, n1 = wc+hs, g0y = ws+hc, n2 = hc-ws
            vtt(GN[:, 0:SW], wcF, hsF, OP.subtract)
            vtt(GN[:, SW:2 * SW], wcF, hsF, OP.add)
            vtt(GN[:, 2 * SW:3 * SW], wsF, hcF, OP.add)
            vtt(GN[:, 3 * SW:4 * SW], hcF, wsF, OP.subtract)
            # clip half-extents of the fixed box [Wc|Hc], /16
            vts(WHc[:, 0:FB], whV[:, 1], 0.5 * SC, None, OP.mult)
            vts(WHc[:, FB:SW], whV[:, 0], 0.5 * SC, None, OP.mult)
            vts(WHc[:, SW:SW + FB], whV[:, 3], 0.5 * SC, None, OP.mult)
            vts(WHc[:, SW + FB:2 * SW], whV[:, 2], 0.5 * SC, None, OP.mult)
            # moving-box bbox half-extents: ex = |wc|+|hs|, ey = |ws|+|hc|
            S.activation(aP1[:], WC2[:], AF.Abs)   # [|wc| | |hc|]
            S.activation(aP2[:], WS2[:], AF.Abs)   # [|ws| | |hs|]
            vtt(EP[:, 0:SW], aP1[:, 0:SW], aP2[:, SW:2 * SW], OP.add)
            vtt(EP[:, SW:2 * SW], aP2[:, 0:SW], aP1[:, SW:2 * SW], OP.add)
            # negated-reciprocal planes rN = -1/d: rNX = [+1/(2wc) | -1/(2hs)],
            # rNY = [+1/(2ws) | +1/(2hc)]; staged f32 as [wc|hs|ws|hc], one
            # wide fast-reciprocal, clamped to +-CL in fp16.
            vts(RP[:, 0:SW], wcF, 2.0, 1e-20, OP.mult, OP.add)
            vts(RP[:, SW:2 * SW], hsF, -2.0, -1e-20, OP.mult, OP.add)
            vts(RP[:, 2 * SW:3 * SW], wsF, 2.0, 1e-20, OP.mult, OP.add)
            vts(RP[:, 3 * SW:4 * SW], hcF, 2.0, 1e-20, OP.mult, OP.add)
            V.reciprocal_approx_fast(out=RP[:], in_=RP[:])
            vts(RN[:, 0:2 * SW], RP[:, 0:2 * SW], CL, -CL, OP.min, OP.max)
            vts(RN[:, 2 * SW:4 * SW], RP[:, 2 * SW:4 * SW], CL, -CL,
                OP.min, OP.max)
            # half-window sizes |Wc*rN| per axis (abs on ACT)
            vtt(v2(MW[:, 0:2 * SW]), bce(WHc[:, 0:SW]),
                v2(RN[:, 0:2 * SW]), OP.mult)
            vtt(v2(MW[:, 2 * SW:4 * SW]), bce(WHc[:, SW:2 * SW]),
                v2(RN[:, 2 * SW:4 * SW]), OP.mult)
            S.activation(AW[:, 0:2 * SW], MW[:, 0:2 * SW], AF.Abs)
            S.activation(AW[:, 2 * SW:4 * SW], MW[:, 2 * SW:4 * SW], AF.Abs)
            # union0 = (w1h1 + w2h2)/1024; *4 to /256 folded into union STT
            vtt(u01[:], WH[:, 0:SW], WH[:, SW:2 * SW], OP.mult)
            u013 = v2(u01[:])
            vtt(union0[:], u013[:, 0], u013[:, 1], OP.add)

            # ---- pre-pass, xy part (lands last) ----
            dd3 = ddS[:].rearrange("p (c h f) -> p c h f", c=2, h=2)
            ddc = ddS[:].rearrange("p (c f) -> p c f", c=2)
            vtt(dd3[:, 0, 0], xyV[:, 0], xyV[:, 1], OP.subtract)  # x1-x2
            vts(dd3[:, 0, 1], dd3[:, 0, 0], -1.0, None, OP.mult)
            vtt(dd3[:, 1, 0], xyV[:, 2], xyV[:, 3], OP.subtract)
            vts(dd3[:, 1, 1], dd3[:, 1, 0], -1.0, None, OP.mult)
            vts(dx16[:], dd3[:, 0, 0], XSC, None, OP.mult)
            vts(dy16[:], dd3[:, 1, 0], XSC, None, OP.mult)
            # delta = R^T * (center difference)/16:
            # aP1 = [csS*ddx | csS*ddy], aP2 = [ssS*ddx | ssS*ddy]
            vtt(v2(aP1[:]), bce(csS[:]), ddc, OP.mult)
            vtt(v2(aP2[:]), bce(ssS[:]), ddc, OP.mult)
            vtt(dXv, aP1[:, 0:SW], aP2[:, SW:2 * SW], OP.add)
            vtt(dYv, aP1[:, SW:2 * SW], aP2[:, 0:SW], OP.subtract)
            vts(nXv, dXv, -1.0, None, OP.mult)
            vts(DM[:, 0:SW], dXv, 2.0, None, OP.mult)
            vts(DM[:, SW:2 * SW], dYv, 2.0, None, OP.mult)

            # corners, edges 0,1 only: ax0 = dX+g0x, ax1 = dX-n1,
            # ay0 = dY+g0y, ay1 = dY+n2
            vtt(AXY[:, 0:SW], dXv, GN[:, 0:SW], OP.add)
            vtt(AXY[:, SW:2 * SW], dXv, GN[:, SW:2 * SW], OP.subtract)
            vtt(AXY[:, 2 * SW:3 * SW], dYv, GN[:, 2 * SW:3 * SW], OP.add)
            vtt(AXY[:, 3 * SW:4 * SW], dYv, GN[:, 3 * SW:4 * SW], OP.add)

            # input tiles no longer needed: free the io pool
            stack.close()

            # ---- slab roots, center form: c01 = ax*rN, c23 = dm*rN - c01 ----
            # CXY = [c01x|c01y | c23x|c23y]; RN/AXY are [x-planes | y-planes]
            vtt(CXY[:, 0:4 * SW], AXY[:], RN[:], OP.mult)
            dmb = bass.AP(DM[:].tensor, DM[:].offset,
                          [list(DM[:].ap[0]), [SW, 2], [0, 2], [1, SW]])
            MM = TLO[:, 0:4 * SW]       # scratch; overwritten by tlo below
            vtt(MM.rearrange("p (a e f) -> p a e f", a=2, e=2), dmb,
                RN[:].rearrange("p (a e f) -> p a e f", a=2, e=2), OP.mult)
            vtt(CXY[:, 4 * SW:8 * SW], CXY[:, 0:4 * SW], MM, OP.subtract)
            # tlo/thi = c -+ aw; AW's [axis|e] layout matches CXY's inner 4SW,
            # broadcast over the e01/e23 pair dim
            awb = bce(AW[:])
            cxy3 = CXY[:].rearrange("p (g f) -> p g f", g=2)
            vtt(v2(TLO[:]), cxy3, awb, OP.subtract)
            vtt(v2(THI[:]), cxy3, awb, OP.add)
            # interval intersect across axes, clamp to [0,1], dt = relu(t1-t0)
            # T0 lives in TLO[0:4SW], T1/TD in THI[0:4SW], DT in THI[4SW:8SW]
            tlo4 = TLO[:].rearrange("p (g a f) -> p g a f", g=2, a=2)
            thi4 = THI[:].rearrange("p (g a f) -> p g a f", g=2, a=2)
            T0 = TLO[:, 0:4 * SW]
            T0v = tlo4[:, :, 0]
            vtt(T0v, tlo4[:, :, 0], tlo4[:, :, 1], OP.max)
            vts(T0v, T0v, 0.0, None, OP.max)
            T1v = thi4[:, :, 0]
            vtt(T1v, thi4[:, :, 0], thi4[:, :, 1], OP.min)
            vts(T1v, T1v, 1.0, None, OP.min)
            vtt(T1v, T1v, T0v, OP.subtract)                   # td in place
            # dt = relu(td) lands contiguous in MW (free after the AW abs)
            DT = MW[:]
            S.activation(v2(DT), T1v, AF.Relu)
            dtg = DT.rearrange("p (g e h f) -> p g e h f", g=2, e=2, h=2)

            # ---- cad = +-cross(center,dir)/2 + wh/1024 (half-cad scale) ----
            # cr/2 = [dY*wc - dX*ws | -dX*hc - dY*hs] -> AW as scratch
            CRa = AW[:, 0:2 * SW]
            CRb = AW[:, 2 * SW:4 * SW]
            vtt(v2(CRa), v2(DXY[:, SW:3 * SW]), v2(WC2[:]), OP.mult)
            vtt(v2(CRb), v2(DXY[:, 0:2 * SW]), v2(WS2[:]), OP.mult)
            vtt(CRa, CRa, CRb, OP.subtract)
            # CAD in TLO[0:4SW] (T0 dead), PC in TLO[4SW:8SW]
            CAD = TLO[:, 0:4 * SW]
            u01b = bce(u01[:])
            vtt(v2(TLO[:, 0:2 * SW]), v2(CRa), u01b, OP.add)
            vtt(v2(TLO[:, 2 * SW:4 * SW]), u01b, v2(CRa), OP.subtract)
            PC = TLO[:, 4 * SW:8 * SW]
            vtt(PC, DT, CAD, OP.mult)
            # piece sums per half: PS lives in AXY[0:2SW] (corners dead)
            PS = AXY[:, 0:2 * SW]
            vtt(PS, TLO[:, 4 * SW:6 * SW], TLO[:, 6 * SW:8 * SW], OP.add)
            psS = sm.tile([P, SW], f16, tag="psS")
            vtt(psS[:], AXY[:, 0:SW], AXY[:, SW:2 * SW], OP.add)
            ps3 = v2(psS[:])

            # ---- enclosing rect (bbox in each frame, min of the two) ----
            # scratch inside CXY (dead after the tlo/thi ops)
            ES1 = CXY[:, 0:2 * SW]
            ES2 = CXY[:, 2 * SW:4 * SW]
            EXT = CXY[:, 4 * SW:6 * SW]
            vtt(ES1, DXY[:, 0:2 * SW], EP[:], OP.add)
            vtt(ES2, EP[:], DXY[:, 0:2 * SW], OP.subtract)
            vtt(ES1, ES1, WHc[:], OP.max)
            vtt(ES2, ES2, WHc[:], OP.max)
            vtt(EXT, ES1, ES2, OP.add)
            exs = sm.tile([P, SW], f16, tag="exs")
            vtt(exs[:], EXT[:, 0:SW], EXT[:, SW:2 * SW], OP.mult)
            es3 = v2(exs[:])
            vtt(area_c[:], es3[:, 0], es3[:, 1], OP.min)

            # ---- SA correction (frame-B half), half-cad scale ----
            sau = sm.tile([P, FB], f16, tag="sau")
            sav = sm.tile([P, FB], f16, tag="sav")
            sax = sm.tile([P, FB], f16, tag="sax")
            say = sm.tile([P, FB], f16, tag="say")
            st1 = sm.tile([P, FB], f16, tag="st1")
            vtt(sau[:], dtg[:, 1, 0, 0], dtg[:, 0, 0, 0], OP.subtract)
            vtt(sav[:], dtg[:, 1, 1, 0], dtg[:, 0, 1, 0], OP.subtract)
            wc_h0 = WC2[:, 0:FB]
            hc_h0 = WC2[:, SW:SW + FB]
            ws_h0 = WS2[:, 0:FB]
            hs_h0 = WS2[:, SW:SW + FB]
            # sax = wc*sau - hs*sav ; say = ws*sau + hc*sav (half of old d2/d3)
            vtt(sax[:], wc_h0, sau[:], OP.mult)
            vtt(st1[:], hs_h0, sav[:], OP.mult)
            vtt(sax[:], sax[:], st1[:], OP.subtract)
            vtt(say[:], ws_h0, sau[:], OP.mult)
            vtt(st1[:], hc_h0, sav[:], OP.mult)
            vtt(say[:], say[:], st1[:], OP.add)
            # corr = ddy/16*(c2*sax - s2*say) - ddx/16*(s2*sax + c2*say)
            c2v = cS[:, 0:FB]
            s2v = sS[:, 0:FB]
            rsx = sm.tile([P, FB], f16, tag="rsx")
            rsy = sm.tile([P, FB], f16, tag="rsy")
            vtt(rsx[:], c2v, sax[:], OP.mult)
            vtt(st1[:], s2v, say[:], OP.mult)
            vtt(rsx[:], rsx[:], st1[:], OP.subtract)
            vtt(rsy[:], s2v, sax[:], OP.mult)
            vtt(st1[:], c2v, say[:], OP.mult)
            vtt(rsy[:], rsy[:], st1[:], OP.add)
            inter16 = sm.tile([P, FB], f16, tag="inter16")
            vtt(inter16[:], dy16[:], rsx[:], OP.mult)
            vtt(st1[:], dx16[:], rsy[:], OP.mult)
            vtt(inter16[:], inter16[:], st1[:], OP.subtract)       # corr
            vtt(inter16[:], inter16[:], ps3[:, 0], OP.add)
            vtt(inter16[:], inter16[:], ps3[:, 1], OP.add)
            inter = sm.tile([P, FB], f32, tag="inter")
            S.activation(inter[:], inter16[:], AF.Relu)  # inter area (/256)

            # ---- final loss (fp32), cubes via one packed ACT Square ----
            union = sm.tile([P, FB], f32, tag="union")
            fr1 = sm.tile([P, FB], f32, tag="fr1")
            IR = sm.tile([P, SW], f32, tag="IR")       # [iou | rr]
            SQ = sm.tile([P, SW], f32, tag="SQ")
            lsa = sm.tile([P, 1], f32, tag="lsa")
            # union = 4*union0 - inter  (the *4 restores the /256 scale)
            V.scalar_tensor_tensor(union[:], union0[:], 4.0, inter[:],
                                   OP.mult, OP.subtract)
            V.reciprocal_approx_fast(out=fr1[:], in_=union[:])
            vtt(IR[:, 0:FB], inter[:], fr1[:], OP.mult)
            vts(IR[:, 0:FB], IR[:, 0:FB], 1e-6, 1.0, OP.max, OP.min)
            V.reciprocal_approx_fast(out=fr1[:], in_=area_c[:])
            vtt(fr1[:], union[:], fr1[:], OP.mult)
            vts(fr1[:], fr1[:], 0.0, 1.0, OP.max, OP.min)
            vts(IR[:, FB:SW], fr1[:], -1.0, 1.0, OP.mult, OP.add)  # 1 - u/ac
            S.activation(SQ[:], IR[:], AF.Square)
            vtt(SQ[:], SQ[:], IR[:], OP.mult)                      # cubes
            cb3 = v2(SQ[:])
            vtt(fr1[:], cb3[:, 0], cb3[:, 1], OP.subtract)         # giou
            V.tensor_reduce(lsa[:], fr1[:], AXL.X, OP.add)         # sum giou
            if debug:
                nc.sync.dma_start(out=dbg_d[0], in_=fr1[:])
                nc.sync.dma_start(out=dbg_d[1], in_=inter[:])
                nc.sync.dma_start(out=dbg_d[2], in_=union[:])
                nc.sync.dma_start(out=dbg_d[3], in_=area_c[:])
            nc.sync.dma_start(out=out_d[:], in_=lsa[:])

    nc.finalize()
    return nc


def _get_nc():
    if "nc" not in _CACHE:
        _CACHE["nc"] = _build()
    return _CACHE["nc"]


def _repack(pred, target):
    """Per-core input repack: planar rows so every SBUF slice is packed.
    ang/wh in fp16; xy quantized to int16 units of 1/32 px (diffs <= ~1500
    units stay exact in fp16). Rows beyond N_CORE are padded with concentric
    axis-aligned boxes whose giou is exactly 1/64 (subtracted on the host)."""
    in_maps = []
    for i in range(N_CORES):
        sl = slice(i * N_CORE, (i + 1) * N_CORE)
        p, t = pred[sl], target[sl]
        ang = np.zeros((2, NPAD), np.float16)
        ang[0, :N_CORE] = p[:, 4]
        ang[1, :N_CORE] = t[:, 4]
        wh = np.empty((4, NPAD), np.float16)
        wh[0, N_CORE:] = 16.0
        wh[1, N_CORE:] = 8.0
        wh[2, N_CORE:] = 16.0
        wh[3, N_CORE:] = 8.0
        wh[0, :N_CORE] = p[:, 2]
        wh[1, :N_CORE] = t[:, 2]
        wh[2, :N_CORE] = p[:, 3]
        wh[3, :N_CORE] = t[:, 3]
        xy = np.full((4, NPAD), 16384, np.int16)
        for r, col in enumerate((p[:, 0], t[:, 0], p[:, 1], t[:, 1])):
            xy[r, :N_CORE] = np.clip(np.rint(col * XQ), 0, 32767).astype(np.int16)

        def lay(a):
            k = a.shape[0]
            return np.ascontiguousarray(
                a.reshape(k, P, FB).transpose(1, 0, 2).reshape(P, k * FB))
        in_maps.append({"ang": lay(ang), "wh": lay(wh), "xy": lay(xy)})
    return in_maps


def kernel(pred, target):
    from concourse.bass_utils import run_bass_kernel_spmd

    pred = np.ascontiguousarray(np.asarray(pred, dtype=np.float32))
    target = np.ascontiguousarray(np.asarray(target, dtype=np.float32))
    nc = _get_nc()
    in_maps = _repack(pred, target)
    res = run_bass_kernel_spmd(nc, in_maps, core_ids=list(range(N_CORES)))
    gsum = np.float64(0.0)
    for i in range(N_CORES):
        gsum += np.asarray(res.results[i]["out"], dtype=np.float64).sum()
    # subtract the exact giou (=1/64) of the concentric pad boxes
    gsum -= float((NPAD - N_CORE) * N_CORES) * 0.015625
    # loss = mean(1 - giou) = 1 - sum(giou)/N
    return np.float32(1.0 - gsum / N_TOTAL)


# revision 9
# speedup vs baseline: 2.5309x; 1.1411x over previous
"""AlphaRotatedGIoULoss on 8 TRN2 NeuronCores.

Data-parallel: 500000 box pairs sharded 62500/core, laid out as
(128 partitions x 489 boxes). Per-box rotated-GIoU via a branchless
line-integral intersection (slab clipping in each box's axis-aligned
frame + a frame-change correction term), so no sorting/gather is needed.

Restructured v2 (vs the 113us session baseline):
- slab roots in center +- half-window form: t = c -+ |Wc*r| with
  pre-negated reciprocal planes, killing the per-edge min/max and subs;
  c23 = dXm*rN - c01 by point symmetry. x and y axes fused into 8u ops.
- cross(corner_e, dir_e)/2 = +-(cross(center,dir)/2) + wh/1024 (the wh
  term is the union's u01 tile), so the full corner planes e2/e3, the
  4SW direction planes, and the 8 ACT copies that built them are gone.
  Everything downstream runs at half-cad scale (final Relu scale 1.0).
- all four reciprocal planes merged into one wide RECIPROCAL_APPROX_FAST
  plus two clamp/cast passes; enclosing-rect x/y stacked into 4u ops.
- tail packs [iou|rr] so one ACT Square + one mult does both cubes.
Heavy chain is fp16 (DVE 2x mode), geometry pre-scaled by 1/16; scratch
tiles are re-used across phases to stay inside SBUF.
"""
import sys
import numpy as np

for _p in ("/opt/trn_rl_repo", "/root/.axon_site/_ro/trn_rl_repo"):
    if _p not in sys.path:
        sys.path.insert(0, _p)

N_CORES = 8
N_TOTAL = 500000
N_CORE = N_TOTAL // N_CORES   # 62500
P = 128                       # all partitions
FB = 489                      # boxes per partition row (128*489 = 62592)
NPAD = P * FB                 # per-core padded count (92 identity pad boxes)
SW = 2 * FB                   # stacked width (both halves)
PI_2 = 1.5707963267948966
SC = 1.0 / 16.0               # global geometry scale (power of 2, exact)
XQ = 32.0                     # xy fixed-point scale (int16 units = px/32)
XSC = SC / XQ                 # folds the xy dequant into the trig scale
CL = 30000.0                  # fp16-safe clamp for reciprocal planes

_CACHE = {}


def _build():
    import concourse.bass as bass
    import concourse.bacc as bacc
    import concourse.tile as tile
    from concourse import mybir

    f32 = mybir.dt.float32
    f16 = mybir.dt.float16
    i16 = mybir.dt.int16
    AF = mybir.ActivationFunctionType
    OP = mybir.AluOpType
    AXL = mybir.AxisListType
    import os
    debug = bool(os.environ.get("K_DEBUG"))
    nc = bacc.Bacc(None, target_bir_lowering=False)
    ang_d = nc.declare_dram_parameter("ang", [P, 2 * FB], f16, isOutput=False)
    wh_d = nc.declare_dram_parameter("wh", [P, 4 * FB], f16, isOutput=False)
    xy_d = nc.declare_dram_parameter("xy", [P, 4 * FB], i16, isOutput=False)
    out_d = nc.declare_dram_parameter("out", [P, 1], f32, isOutput=True)
    dbg_d = None
    if debug:
        dbg_d = nc.declare_dram_parameter("dbg", [4, P, FB], f32, isOutput=True)

    V = nc.vector
    S = nc.scalar

    def vtt(out, a, b, op):
        V.tensor_tensor(out, a, b, op)

    def vts(out, in_, s1, s2, op0, op1=None):
        if op1 is None:
            V.tensor_scalar(out, in_, s1, None, op0)
        else:
            V.tensor_scalar(out, in_, s1, s2, op0, op1)

    def bce(apv, n=2, axis=1):
        # stride-0 broadcast: insert a [0, n] dim at `axis` (after partition)
        ap_l = [list(d) for d in apv.ap]
        ap_l.insert(axis, [0, n])
        return bass.AP(apv.tensor, apv.offset, ap_l)

    def v2(ap, h=2):
        return ap.rearrange("p (h f) -> p h f", h=h)

    from contextlib import ExitStack

    with tile.TileContext(nc) as tc:
        with (
            tc.tile_pool(name="pre", bufs=1) as pre,
            tc.tile_pool(name="small", bufs=1) as sm,
            ExitStack() as stack,
        ):
            io = stack.enter_context(tc.tile_pool(name="io", bufs=1))
            angT = io.tile([P, 2 * FB], f16, tag="angT")
            whT = io.tile([P, 4 * FB], f16, tag="whT")
            xyT = io.tile([P, 4 * FB], i16, tag="xyT")
            pio2 = sm.tile([P, 1], f32, tag="pio2")
            V.memset(pio2[:], PI_2)
            # 1-elem warm-up: loads the Sin ACT table while the DMA runs
            warm = sm.tile([P, 1], f32, tag="warm")
            S.activation(warm[:], pio2[:], AF.Sin)
            angV = angT[:].rearrange("p (h f) -> p h f", h=2)
            whV = whT[:].rearrange("p (c f) -> p c f", c=4)   # w1,w2,h1,h2
            xyV = xyT[:].rearrange("p (c f) -> p c f", c=4)   # x1,x2,y1,y2
            # host pre-shuffles inputs into these exact SBUF layouts, so each
            # partition line is one fully-contiguous DMA descriptor.
            # angles first (small, unblocks the Sin chain), then wh, then xy
            nc.sync.dma_start(out=angT[:], in_=ang_d[:])
            nc.sync.dma_start(out=whT[:], in_=wh_d[:])
            nc.sync.dma_start(out=xyT[:], in_=xy_d[:])

            def T(name, units, dt=f16):
                # `units` in FB-widths
                return pre.tile([P, units * FB], dt, name=name, tag=name)

            # --- tiles (persistent + phase-reused scratch) ---
            dlt, dltw = T("dlt", 2, f32), T("dltw", 2, f32)
            cdsd = T("cdsd", 4)                   # [cd | sd] planes
            cS, sS = T("cS", 2), T("sS", 2)       # [c2|c1], [s2|s1]
            csS, ssS = T("csS", 2), T("ssS", 2)
            WH = T("WH", 4)                       # [whS | hhS]
            WC2 = T("WC2", 4)                     # [wc | hc]
            WS2 = T("WS2", 4)                     # [ws | hs]
            WHc = T("WHc", 4)                     # [Wc | Hc] clip half-extents
            GN = T("GN", 8)                       # [g0x | n1 | g0y | n2]
            AXY = T("AXY", 8)                     # corners [ax0|ax1|ay0|ay1]
            RP = T("RP", 8, f32)                  # recip staging [wc|hs|ws|hc]
            RN = T("RN", 8)                       # [rNX(e0,e1) | rNY(e0,e1)]
            MW = T("MW", 8)                       # [Wc*rNX | Hc*rNY] -> scratch
            AW = T("AW", 8)                       # |MW|
            CXY = T("CXY", 16)                    # [c01xy | c23xy]
            TLO = T("TLO", 16)                    # tlo; later T0/CAD/PC
            THI = T("THI", 16)                    # thi; later T1/TD/DT
            DXY = T("DXY", 4)                     # [dX | dY]
            DM = T("DM", 4)                       # [2dX | 2dY]
            ddS = T("ddS", 4)                     # [ddx | ddy]
            aP1, aP2 = T("aP1", 4), T("aP2", 4)
            EP = T("EP", 4)                       # [exP | eyP]
            u01 = sm.tile([P, SW], f16, tag="u01")
            union0 = sm.tile([P, FB], f32, tag="union0")
            area_c = sm.tile([P, FB], f32, tag="area_c")

            dXv = DXY[:, 0 * SW:1 * SW]
            dYv = DXY[:, 1 * SW:2 * SW]
            wcF = WC2[:, 0:SW]
            hcF = WC2[:, SW:2 * SW]
            wsF = WS2[:, 0:SW]
            hsF = WS2[:, SW:2 * SW]
            cdS = cdsd[:, 0:SW]
            sdS = cdsd[:, SW:2 * SW]

            # ---- pre-pass, angle part (only needs angT) ----
            dlt3 = v2(dlt[:])
            vtt(dlt3[:, 0], angV[:, 0], angV[:, 1], OP.subtract)  # a1-a2 (f32)
            vts(dlt3[:, 1], dlt3[:, 0], -1.0, None, OP.mult)
            S.activation(cS[:, 0:FB], angV[:, 1], AF.Sin, bias=pio2[:])   # c2
            S.activation(cS[:, FB:SW], angV[:, 0], AF.Sin, bias=pio2[:])  # c1
            S.activation(sS[:, 0:FB], angV[:, 1], AF.Sin)                 # s2
            S.activation(sS[:, FB:SW], angV[:, 0], AF.Sin)                # s1
            S.activation(sdS, dlt[:], AF.Sin)                    # [sd|-sd]
            # cos(dlt) = sin(dlt + pi/2); wrap into [-pi, pi] first
            V.add_range_wrap(dltw[:], dlt[:], PI_2, 3.141592653589793,
                             6.283185307179586)
            S.activation(cdS, dltw[:], AF.Sin)                   # [cd|cd]
            # scaled trig copies carry geometry scale + xy dequant into dX/dY
            vts(csS[:], cS[:], XSC, None, OP.mult)
            vts(ssS[:], sS[:], XSC, None, OP.mult)

            # ---- pre-pass, wh part ----
            vts(WH[:, 0:SW], whV[:, 0:2], 0.5 * SC, None, OP.mult)       # whS
            vts(WH[:, SW:2 * SW], whV[:, 2:4], 0.5 * SC, None, OP.mult)  # hhS
            WH3 = v2(WH[:])
            # [wc|hc] = [whS|hhS]*cd ; [ws|hs] = [whS|hhS]*sd
            vtt(v2(WC2[:]), WH3, bce(cdS), OP.mult)
            vtt(v2(WS2[:]), WH3, bce(sdS), OP.mult)
            # corner offsets: g0x = wc-hs, n1 = wc+hs, g0y = ws+hc, n2 = hc-ws
            vtt(GN[:, 0:SW], wcF, hsF, OP.subtract)
            vtt(GN[:, SW:2 * SW], wcF, hsF, OP.add)
            vtt(GN[:, 2 * SW:3 * SW], wsF, hcF, OP.add)
            vtt(GN[:, 3 * SW:4 * SW], hcF, wsF, OP.subtract)
            # clip half-extents of the fixed box [Wc|Hc], /16
            vts(WHc[:, 0:FB], whV[:, 1], 0.5 * SC, None, OP.mult)
            vts(WHc[:, FB:SW], whV[:, 0], 0.5 * SC, None, OP.mult)
            vts(WHc[:, SW:SW + FB], whV[:, 3], 0.5 * SC, None, OP.mult)
            vts(WHc[:, SW + FB:2 * SW], whV[:, 2], 0.5 * SC, None, OP.mult)
            # moving-box bbox half-extents: ex = |wc|+|hs|, ey = |ws|+|hc|
            S.activation(aP1[:], WC2[:], AF.Abs)   # [|wc| | |hc|]
            S.activation(aP2[:], WS2[:], AF.Abs)   # [|ws| | |hs|]
            vtt(EP[:, 0:SW], aP1[:, 0:SW], aP2[:, SW:2 * SW], OP.add)
            vtt(EP[:, SW:2 * SW], aP2[:, 0:SW], aP1[:, SW:2 * SW], OP.add)
            # negated-reciprocal planes rN = -1/d: rNX = [+1/(2wc) | -1/(2hs)],
            # rNY = [+1/(2ws) | +1/(2hc)]; staged f32 as [wc|hs|ws|hc], one
            # wide fast-reciprocal, clamped to +-CL in fp16.
            vts(RP[:, 0:SW], wcF, 2.0, 1e-20, OP.mult, OP.add)
            vts(RP[:, SW:2 * SW], hsF, -2.0, -1e-20, OP.mult, OP.add)
            vts(RP[:, 2 * SW:3 * SW], wsF, 2.0, 1e-20, OP.mult, OP.add)
            vts(RP[:, 3 * SW:4 * SW], hcF, 2.0, 1e-20, OP.mult, OP.add)
            V.reciprocal_approx_fast(out=RP[:], in_=RP[:])
            vts(RN[:], RP[:], CL, -CL, OP.min, OP.max)
            # half-window sizes |Wc*rN| per axis (abs on ACT)
            whcb = bass.AP(WHc[:].tensor, WHc[:].offset,
                           [list(WHc[:].ap[0]), [SW, 2], [0, 2], [1, SW]])
            vtt(MW[:].rearrange("p (a e f) -> p a e f", a=2, e=2), whcb,
                RN[:].rearrange("p (a e f) -> p a e f", a=2, e=2), OP.mult)
            S.activation(AW[:], MW[:], AF.Abs)
            # union0 = (w1h1 + w2h2)/1024; *4 to /256 folded into union STT
            vtt(u01[:], WH[:, 0:SW], WH[:, SW:2 * SW], OP.mult)
            u013 = v2(u01[:])
            vtt(union0[:], u013[:, 0], u013[:, 1], OP.add)

            # ---- pre-pass, xy part (lands last) ----
            dd3 = ddS[:].rearrange("p (c h f) -> p c h f", c=2, h=2)
            ddc = ddS[:].rearrange("p (c f) -> p c f", c=2)
            vtt(dd3[:, 0, 0], xyV[:, 0], xyV[:, 1], OP.subtract)  # x1-x2
            vts(dd3[:, 0, 1], dd3[:, 0, 0], -1.0, None, OP.mult)
            vtt(dd3[:, 1, 0], xyV[:, 2], xyV[:, 3], OP.subtract)
            vts(dd3[:, 1, 1], dd3[:, 1, 0], -1.0, None, OP.mult)
            # delta = R^T * (center difference)/16:
            # aP1 = [csS*ddx | csS*ddy], aP2 = [ssS*ddx | ssS*ddy]
            vtt(v2(aP1[:]), bce(csS[:]), ddc, OP.mult)
            vtt(v2(aP2[:]), bce(ssS[:]), ddc, OP.mult)
            vtt(dXv, aP1[:, 0:SW], aP2[:, SW:2 * SW], OP.add)
            vtt(dYv, aP1[:, SW:2 * SW], aP2[:, 0:SW], OP.subtract)
            vts(DM[:, 0:SW], dXv, 2.0, None, OP.mult)
            vts(DM[:, SW:2 * SW], dYv, 2.0, None, OP.mult)

            # corners, edges 0,1 only: ax0 = dX+g0x, ax1 = dX-n1,
            # ay0 = dY+g0y, ay1 = dY+n2
            vtt(AXY[:, 0:SW], dXv, GN[:, 0:SW], OP.add)
            vtt(AXY[:, SW:2 * SW], dXv, GN[:, SW:2 * SW], OP.subtract)
            vtt(AXY[:, 2 * SW:3 * SW], dYv, GN[:, 2 * SW:3 * SW], OP.add)
            vtt(AXY[:, 3 * SW:4 * SW], dYv, GN[:, 3 * SW:4 * SW], OP.add)

            # input tiles no longer needed: free the io pool
            stack.close()

            # ---- slab roots, center form: c01 = ax*rN, c23 = dm*rN - c01 ----
            # CXY = [c01x|c01y | c23x|c23y]; RN/AXY are [x-planes | y-planes]
            vtt(CXY[:, 0:4 * SW], AXY[:], RN[:], OP.mult)
            dmb = bass.AP(DM[:].tensor, DM[:].offset,
                          [list(DM[:].ap[0]), [SW, 2], [0, 2], [1, SW]])
            MM = TLO[:, 0:4 * SW]       # scratch; overwritten by tlo below
            vtt(MM.rearrange("p (a e f) -> p a e f", a=2, e=2), dmb,
                RN[:].rearrange("p (a e f) -> p a e f", a=2, e=2), OP.mult)
            vtt(CXY[:, 4 * SW:8 * SW], CXY[:, 0:4 * SW], MM, OP.subtract)
            # tlo/thi = c -+ aw; AW's [axis|e] layout matches CXY's inner 4SW,
            # broadcast over the e01/e23 pair dim
            awb = bce(AW[:])
            cxy3 = CXY[:].rearrange("p (g f) -> p g f", g=2)
            vtt(v2(TLO[:]), cxy3, awb, OP.subtract)
            vtt(v2(THI[:]), cxy3, awb, OP.add)
            # interval intersect across axes, clamp to [0,1], dt = relu(t1-t0)
            # T0 lives in TLO[0:4SW], T1/TD in THI[0:4SW], DT in THI[4SW:8SW]
            tlo4 = TLO[:].rearrange("p (g a f) -> p g a f", g=2, a=2)
            thi4 = THI[:].rearrange("p (g a f) -> p g a f", g=2, a=2)
            T0 = TLO[:, 0:4 * SW]
            T0v = tlo4[:, :, 0]
            vtt(T0v, tlo4[:, :, 0], tlo4[:, :, 1], OP.max)
            vts(T0v, T0v, 0.0, None, OP.max)
            T1v = thi4[:, :, 0]
            vtt(T1v, thi4[:, :, 0], thi4[:, :, 1], OP.min)
            vts(T1v, T1v, 1.0, None, OP.min)
            vtt(T1v, T1v, T0v, OP.subtract)                   # td in place
            # dt = relu(td) lands contiguous in MW (free after the AW abs)
            DT = MW[:]
            S.activation(v2(DT), T1v, AF.Relu)
            dtg = DT.rearrange("p (g e h f) -> p g e h f", g=2, e=2, h=2)

            # ---- inter via rebased origins: the h0 half's per-edge cad is
            # the constant u01 (origin at the moving box's own center), so
            # inter = u01*sum(dt) per half + the h1 cross terms
            # sum dt over pair and edge dims -> S_dt per (h, box)
            vtt(AXY[:, 0:2 * SW], DT[:, 0:2 * SW], DT[:, 2 * SW:4 * SW],
                OP.add)
            sdt = sm.tile([P, SW], f16, tag="sdt")
            vtt(sdt[:], AXY[:, 0:SW], AXY[:, SW:2 * SW], OP.add)
            bse = sm.tile([P, SW], f16, tag="bse")
            vtt(bse[:], u01[:], sdt[:], OP.mult)
            bse3 = v2(bse[:])
            # h1 cross terms: crA = dY*wc - dX*ws, crBn = dX*hc + dY*hs
            dX_h1 = DXY[:, FB:SW]
            dY_h1 = DXY[:, SW + FB:2 * SW]
            wc_h1 = WC2[:, FB:SW]
            hc_h1 = WC2[:, SW + FB:2 * SW]
            ws_h1 = WS2[:, FB:SW]
            hs_h1 = WS2[:, SW + FB:2 * SW]
            crA = sm.tile([P, FB], f16, tag="crA")
            crBn = sm.tile([P, FB], f16, tag="crBn")
            st1 = sm.tile([P, FB], f16, tag="st1")
            st2 = sm.tile([P, FB], f16, tag="st2")
            vtt(crA[:], dY_h1, wc_h1, OP.mult)
            vtt(st1[:], dX_h1, ws_h1, OP.mult)
            vtt(crA[:], crA[:], st1[:], OP.subtract)
            vtt(crBn[:], dX_h1, hc_h1, OP.mult)
            vtt(st1[:], dY_h1, hs_h1, OP.mult)
            vtt(crBn[:], crBn[:], st1[:], OP.add)
            # du = dt_e0 - dt_e2, dv = dt_e1 - dt_e3 (h1 planes)
            vtt(st1[:], dtg[:, 0, 0, 1], dtg[:, 1, 0, 1], OP.subtract)
            vtt(st2[:], dtg[:, 0, 1, 1], dtg[:, 1, 1, 1], OP.subtract)
            vtt(crA[:], crA[:], st1[:], OP.mult)
            vtt(crBn[:], crBn[:], st2[:], OP.mult)
            inter16 = sm.tile([P, FB], f16, tag="inter16")
            vtt(inter16[:], bse3[:, 0], bse3[:, 1], OP.add)
            vtt(inter16[:], inter16[:], crA[:], OP.add)
            vtt(inter16[:], inter16[:], crBn[:], OP.subtract)

            # ---- enclosing rect (bbox in each frame, min of the two) ----
            # scratch inside CXY (dead after the tlo/thi ops)
            ES1 = CXY[:, 0:2 * SW]
            ES2 = CXY[:, 2 * SW:4 * SW]
            EXT = CXY[:, 4 * SW:6 * SW]
            vtt(ES1, DXY[:, 0:2 * SW], EP[:], OP.add)
            vtt(ES2, EP[:], DXY[:, 0:2 * SW], OP.subtract)
            vtt(ES1, ES1, WHc[:], OP.max)
            vtt(ES2, ES2, WHc[:], OP.max)
            vtt(EXT, ES1, ES2, OP.add)
            exs = sm.tile([P, SW], f16, tag="exs")
            vtt(exs[:], EXT[:, 0:SW], EXT[:, SW:2 * SW], OP.mult)
            es3 = v2(exs[:])
            vtt(area_c[:], es3[:, 0], es3[:, 1], OP.min)

            inter = sm.tile([P, FB], f32, tag="inter")
            S.activation(inter[:], inter16[:], AF.Relu)  # inter area (/256)

            # ---- final loss (fp32), cubes via one packed ACT Square ----
            union = sm.tile([P, FB], f32, tag="union")
            fr1 = sm.tile([P, FB], f32, tag="fr1")
            IR = sm.tile([P, SW], f32, tag="IR")       # [iou | rr]
            SQ = sm.tile([P, SW], f32, tag="SQ")
            lsa = sm.tile([P, 1], f32, tag="lsa")
            # union = 4*union0 - inter  (the *4 restores the /256 scale)
            V.scalar_tensor_tensor(union[:], union0[:], 4.0, inter[:],
                                   OP.mult, OP.subtract)
            V.reciprocal_approx_fast(out=fr1[:], in_=union[:])
            vtt(IR[:, 0:FB], inter[:], fr1[:], OP.mult)
            vts(IR[:, 0:FB], IR[:, 0:FB], 1e-6, 1.0, OP.max, OP.min)
            V.reciprocal_approx_fast(out=fr1[:], in_=area_c[:])
            vtt(fr1[:], union[:], fr1[:], OP.mult)
            vts(fr1[:], fr1[:], 0.0, 1.0, OP.max, OP.min)
            vts(IR[:, FB:SW], fr1[:], -1.0, 1.0, OP.mult, OP.add)  # 1 - u/ac
            S.activation(SQ[:], IR[:], AF.Square)
            vtt(SQ[:], SQ[:], IR[:], OP.mult)                      # cubes
            cb3 = v2(SQ[:])
            vtt(fr1[:], cb3[:, 0], cb3[:, 1], OP.subtract)         # giou
            V.tensor_reduce(lsa[:], fr1[:], AXL.X, OP.add)         # sum giou
            if debug:
                nc.sync.dma_start(out=dbg_d[0], in_=fr1[:])
                nc.sync.dma_start(out=dbg_d[1], in_=inter[:])
                nc.sync.dma_start(out=dbg_d[2], in_=union[:])
                nc.sync.dma_start(out=dbg_d[3], in_=area_c[:])
            nc.sync.dma_start(out=out_d[:], in_=lsa[:])

    nc.finalize()
    return nc


def _get_nc():
    if "nc" not in _CACHE:
        _CACHE["nc"] = _build()
    return _CACHE["nc"]


def _repack(pred, target):
    """Per-core input repack: planar rows so every SBUF slice is packed.
    ang/wh in fp16; xy quantized to int16 units of 1/32 px (diffs <= ~1500
    units stay exact in fp16). Rows beyond N_CORE are padded with concentric
    axis-aligned boxes whose giou is exactly 1/64 (subtracted on the host)."""
    in_maps = []
    for i in range(N_CORES):
        sl = slice(i * N_CORE, (i + 1) * N_CORE)
        p, t = pred[sl], target[sl]
        ang = np.zeros((2, NPAD), np.float16)
        ang[0, :N_CORE] = p[:, 4]
        ang[1, :N_CORE] = t[:, 4]
        wh = np.empty((4, NPAD), np.float16)
        wh[0, N_CORE:] = 16.0
        wh[1, N_CORE:] = 8.0
        wh[2, N_CORE:] = 16.0
        wh[3, N_CORE:] = 8.0
        wh[0, :N_CORE] = p[:, 2]
        wh[1, :N_CORE] = t[:, 2]
        wh[2, :N_CORE] = p[:, 3]
        wh[3, :N_CORE] = t[:, 3]
        xy = np.full((4, NPAD), 16384, np.int16)
        for r, col in enumerate((p[:, 0], t[:, 0], p[:, 1], t[:, 1])):
            xy[r, :N_CORE] = np.clip(np.rint(col * XQ), 0, 32767).astype(np.int16)

        def lay(a):
            k = a.shape[0]
            return np.ascontiguousarray(
                a.reshape(k, P, FB).transpose(1, 0, 2).reshape(P, k * FB))
        in_maps.append({"ang": lay(ang), "wh": lay(wh), "xy": lay(xy)})
    return in_maps


def kernel(pred, target):
    from concourse.bass_utils import run_bass_kernel_spmd

    pred = np.ascontiguousarray(np.asarray(pred, dtype=np.float32))
    target = np.ascontiguousarray(np.asarray(target, dtype=np.float32))
    nc = _get_nc()
    in_maps = _repack(pred, target)
    res = run_bass_kernel_spmd(nc, in_maps, core_ids=list(range(N_CORES)))
    gsum = np.float64(0.0)
    for i in range(N_CORES):
        gsum += np.asarray(res.results[i]["out"], dtype=np.float64).sum()
    # subtract the exact giou (=1/64) of the concentric pad boxes
    gsum -= float((NPAD - N_CORE) * N_CORES) * 0.015625
    # loss = mean(1 - giou) = 1 - sum(giou)/N
    return np.float32(1.0 - gsum / N_TOTAL)
